# revision 58
# baseline (speedup 1.0000x reference)
"""Multi-head attention (B=4, N=2048, C=1024, H=16, D=64) on 8 trn2 cores.

Sharding: core c = (batch b = c//2, head-half g = c%2). Each core computes
attention for 8 heads of one batch plus the partial output projection over
its 512 channels; the host sums the two partials per batch and adds b_proj.

Device layout (all matmul operands bf16, fp32 PSUM accumulation):
  - host passes xT = x[b].T                       [1024, 2048]
  - QK^T:  qkT[m, n]  = wqk[:, m].T @ xT          (lhsT=wqk, rhs=xT)
  - V:     v[n, vc]   = xT[:, n].T @ wv           (lhsT=xT,  rhs=wv)
           stored interleaved as [V_h | ones] blocks of 65 per head
  - S^T:   s[m, n]    = kT[:, m].T @ qT[:, n]     (per head, K = d = 64)
           PAIRED: heads 2p / 2p+1 sit at SBUF partitions 0:64 / 64:128 of
           one qkT tile, so their K=64 matmuls land on disjoint PE
           row-groups (tile_position (0,0) vs (64,0)) and DIFFERENT PSUM
           banks of one sp tile; the PE's reorder window overlaps them,
           nearly doubling S throughput (measured ~60-100us win).
  - E = exp(s / 8) via ScalarE (scores are O(1): no max subtraction),
    one [128, 1024] activation covers both paired heads
  - PV:    u[dv, n]   = V1[m, dv].T @ E[m, n]     row 64 = softmax denom
  - norm:  attnT = u[0:64] * broadcast(1/u[64])   (PE K=1 broadcast matmul)
  - proj:  out[n, oc] = attnT[:, n].T @ wp        partial over 512 channels

Rejected on measurement (see CONFIG): fp8e4 DoubleRow for S/PV (LDWEIGHTS
penalty without Fast Weight Load loses more than the 0.5 cyc/row gains)
and a DVE/GpSimd polynomial-exp offload (engine overheads exceed the
ScalarE relief).
"""

import numpy as np
import ml_dtypes

B, N, C = 4, 2048, 1024
H, D = 16, 64
HPC = 8            # heads per core
QKC = HPC * D      # 512 q (and k, v) channels per core
NCORES = 8

# exp(u/16) ~ (y^4 + PC0 y^3 + PC1 y^2 + PC2 y + PC3)^2, y = PALPHA * u
# (fit over u in [-32, 32]; PSUM scores u = 2*q8.k8)
PALPHA = float(24 ** -0.25 / 32)
PCOEF = (1.92592954, 2.45691537, 2.20735042, 0.99973144)  # c0..c3
# key-tiles (mt) whose exp runs on DVE+GpSimd instead of ScalarE
DVE_SET = frozenset((4, 9, 14))

# kernel variant switches (bisection/tuning).  Measured on HW (differential
# wall-clock, relative): s_dr (fp8 DoubleRow S), pv_dr (fp8 PV w/ residual
# planes) and dve_set (poly-exp offload) all regressed.  Ship:
#  - pair_s: S matmuls of heads 2p/2p+1 issued back-to-back into disjoint
#    PE row-groups + PSUM banks (overlap; ~60-100us win)
#  - pv_mdr: fp8e4 PV DoubleRow contracting TWO key-tiles per instruction
#    (halves PV rows AND ldweights count; ~30-45us win, rel err 1.49e-2)
CONFIG = dict(s_dr=False, pv_dr=False, dve_set=(), pair_s=True, pv_mdr=True)

_nc_cache = None


def build_nc(split_waits=True, repeat=1, config=None):
    import concourse.bass as bass
    import concourse.mybir as mybir
    import concourse.tile as tile

    cfg = dict(CONFIG)
    if config:
        cfg.update(config)
    s_dr = cfg["s_dr"]
    pv_dr = cfg["pv_dr"]
    pair_s = cfg.get("pair_s", False)
    dve_set = frozenset(cfg["dve_set"])
    # timing-only probes (WRONG numerics): halve one engine's work to see
    # whether total time follows it (bottleneck identification)
    probe = cfg.get("probe", None)
    early_proj = cfg.get("early_proj", False)
    dma_norm = cfg.get("dma_norm", False)
    # fp8 PV DoubleRow with REAL m-tile planes (2 key-tiles contracted per
    # instruction): halves PV matmul rows AND ldweights count
    pv_mdr = cfg.get("pv_mdr", False)
    assert not (pair_s and s_dr)
    assert not (pv_mdr and (pv_dr or not pair_s))

    bf16 = mybir.dt.bfloat16
    f32 = mybir.dt.float32
    f32r = mybir.dt.float32r
    f8 = mybir.dt.float8e4
    DR = mybir.MatmulPerfMode.DoubleRow
    qk_dt = f8 if s_dr else bf16
    e_dt = f8 if (pv_dr or pv_mdr) else bf16

    def plane_ap(ap, stride):
        """Insert a [stride, 2] plane dim after the partition dim (DoubleRow
        operand).  stride=0 contracts the same data twice (result x2);
        a real stride contracts two adjacent tiles in one pass."""
        return bass.AP(
            tensor=ap.tensor, offset=ap.offset,
            ap=[list(ap.ap[0]), [stride, 2]] + [list(d) for d in ap.ap[1:]],
        )

    def dup_plane(ap):
        return plane_ap(ap, 0)

    nc = bass.Bass()
    xT_d = nc.declare_dram_parameter("xT", [C, N], bf16, isOutput=False)
    wqk_d = nc.declare_dram_parameter("wqk", [C, 2 * QKC], bf16, isOutput=False)
    wv_d = nc.declare_dram_parameter("wv", [C, QKC], bf16, isOutput=False)
    wp_d = nc.declare_dram_parameter("wp", [QKC, C], bf16, isOutput=False)
    out_d = nc.declare_dram_parameter("out", [N, C], f32, isOutput=True)

    rdram = nc.dram_tensor("rscratch", [32, 512], bf16)

    KT = C // 128          # 8 contraction tiles for qkv projection
    NT = N // 128          # 16 seq tiles
    NB = N // 512          # 4 seq blocks of 512
    QKT = 2 * QKC // 128   # 8 qk-channel tiles

    with tile.TileContext(nc) as tc:
        with (
            tc.tile_pool(name="big", bufs=1) as big,
            tc.tile_pool(name="work",
                         bufs=18 if pv_dr else (16 if dve_set else 18)) as workp,
            tc.tile_pool(name="poly", bufs=1) as polyp,
            tc.tile_pool(name="outp", bufs=4) as outp,
            tc.tile_pool(name="small", bufs=4) as smallp,
            tc.tile_pool(name="mm", bufs=2, space="PSUM") as mmp,
            tc.tile_pool(name="spsum", bufs=2, space="PSUM") as spsum,
            tc.tile_pool(name="pvpsum", bufs=2, space="PSUM") as pvpsum,
        ):
            # ---- load inputs ----
            xt = big.tile([128, KT, N], bf16, tag="xt")
            wqk = big.tile([128, KT, 2 * QKC], bf16, tag="wqk")
            wv = big.tile([128, KT, QKC], bf16, tag="wv")
            wp = big.tile([128, QKC // 128, C], bf16, tag="wp")
            xT_r = xT_d.rearrange("(t p) n -> p t n", p=128)
            wqk_r = wqk_d.rearrange("(t p) m -> p t m", p=128)
            wv_r = wv_d.rearrange("(t p) m -> p t m", p=128)
            wp_r = wp_d.rearrange("(t p) m -> p t m", p=128)
            # Each DMA instruction costs ~650ns of serialized issue on the
            # sync sequencer, so use few, large DMAs in consumption order.
            # wqk is host-reordered pair-major ([q|k] 256-col block per head
            # pair) so one DMA loads exactly what head pair 0 needs first.
            # leading chunks split in half so the first qk group's kt=0..3
            # matmuls start while the second half still streams
            nc.sync.dma_start(out=wqk[:, 0:4, 0:256], in_=wqk_r[:, 0:4, 0:256])
            nc.sync.dma_start(out=xt[:, 0:4, 0:512], in_=xT_r[:, 0:4, 0:512])
            nc.sync.dma_start(out=wqk[:, 4:8, 0:256], in_=wqk_r[:, 4:8, 0:256])
            nc.sync.dma_start(out=xt[:, 4:8, 0:512], in_=xT_r[:, 4:8, 0:512])
            for nb in range(1, NB):
                nc.sync.dma_start(out=xt[:, :, nb * 512:(nb + 1) * 512],
                                  in_=xT_r[:, :, nb * 512:(nb + 1) * 512])
            nc.sync.dma_start(out=wv, in_=wv_r)
            for pr in range(1, 4):
                nc.sync.dma_start(out=wqk[:, :, pr * 256:(pr + 1) * 256],
                                  in_=wqk_r[:, :, pr * 256:(pr + 1) * 256])
            nc.sync.dma_start(out=wp, in_=wp_r)

            ones = big.tile([1, 64], bf16, tag="ones")
            nc.vector.memset(ones, 1.0)
            # dummy exp right after DMA issue: walrus attaches the ~2.7us
            # ACT_TABLE_LOAD here, hiding it under the input-DMA wait instead
            # of stalling the first real softmax exp
            warm = big.tile([1, 16], f32, tag="actwarm")
            nc.vector.memset(warm, 0.0)
            nc.scalar.activation(out=warm, in_=warm,
                                 func=mybir.ActivationFunctionType.Exp,
                                 scale=1.0)

            # ---- persistent intermediates ----
            qkT = [big.tile([128, N], qk_dt, tag=f"qkT{i}", name=f"qkT{i}") for i in range(QKT)]
            if pv_dr:
                # V DoubleRow planes: [128, plane 2, head 8 x 68] fp8
                # (per head: cols 0..63 = V8 / V-V8 residual, col 64 = 1 / 0)
                v1 = [big.tile([128, 2, HPC * 68], f8, tag=f"v1_{i}", name=f"v1_{i}") for i in range(NT)]
            elif pv_mdr:
                # V DoubleRow m-planes: tile t holds key-tiles 2t (plane 0)
                # and 2t+1 (plane 1); col 64 = 1 in BOTH planes (denominator
                # sums over both key-tiles)
                v1 = [big.tile([128, 2, HPC * 68], f8, tag=f"v1_{i}", name=f"v1_{i}") for i in range(NT // 2)]
            else:
                v1 = [big.tile([128, HPC * 65], bf16, tag=f"v1_{i}", name=f"v1_{i}") for i in range(NT)]
            attnT = [big.tile([128, N], bf16, tag=f"attnT{i}", name=f"attnT{i}") for i in range(4)]

            def qk_group(mt, nb):
                # wqk is pair-major: q cols of pair p at p*256, k at p*256+128
                co = (mt % 4) * 256 + (mt // 4) * 128
                ps = mmp.tile([128, 512], f32, tag="mm")
                for kt in range(KT):
                    nc.tensor.matmul(
                        ps,
                        lhsT=wqk[:, kt, co:co + 128],
                        rhs=xt[:, kt, nb * 512:(nb + 1) * 512],
                        start=(kt == 0),
                        stop=(kt == KT - 1),
                    )
                nc.vector.tensor_copy(
                    out=qkT[mt][:, nb * 512:(nb + 1) * 512], in_=ps
                )

            def qk_tile(mt):
                """qkT[mt] = (wqk[:, mt*128:+128]).T @ xT  -> [128, 2048]"""
                for nb in range(NB):
                    qk_group(mt, nb)

            def v_tile(nt):
                """v1[nt][:, pl, h*68:+64] = (V8, V-V8) planes; col 64 = (1,0)"""
                ps = mmp.tile([128, 512], f32, tag="mm")
                for kt in range(KT):
                    nc.tensor.matmul(
                        ps,
                        lhsT=xt[:, kt, nt * 128:(nt + 1) * 128],
                        rhs=wv[:, kt, :],
                        start=(kt == 0),
                        stop=(kt == KT - 1),
                    )
                psr = ps.rearrange("p (h e) -> p h e", e=64)
                if pv_dr:
                    v3 = v1[nt].rearrange("p t (h e) -> p t h e", e=68)
                    nc.vector.memset(v3[:, 0, :, 64:65], 1.0)
                    nc.vector.memset(v3[:, 1, :, 64:65], 0.0)
                    nc.vector.tensor_copy(out=v3[:, 0, :, 0:64], in_=psr)
                    nc.vector.tensor_sub(out=v3[:, 1, :, 0:64], in0=psr,
                                         in1=v3[:, 0, :, 0:64])
                elif pv_mdr:
                    v3 = v1[nt // 2].rearrange("p t (h e) -> p t h e", e=68)
                    pl = nt % 2
                    nc.vector.memset(v3[:, pl, :, 64:65], 1.0)
                    nc.vector.tensor_copy(out=v3[:, pl, :, 0:64], in_=psr)
                else:
                    v3 = v1[nt].rearrange("p (h e) -> p h e", e=65)
                    nc.vector.memset(v3[:, :, 64:65], 1.0)
                    nc.vector.tensor_copy(out=v3[:, :, 0:64], in_=psr)

            rb_idx = [0]

            def s_units_pair(blk, fillers, dense=False):
                """Pair mode: block (p, ng, half) runs heads 2p (PE rows 0:64)
                and 2p+1 (rows 64:128) back-to-back into the two banks of one
                sp tile — disjoint row-groups + banks, so the PE overlaps
                them.  One exp covers both heads."""
                p, ng, half = blk
                fillers = list(fillers)
                if dense:
                    slots = set(range(NT))
                elif fillers:
                    stride = max(1, NT // len(fillers))
                    slots = set(range(stride - 1, NT, stride))
                else:
                    slots = set()
                qt = qkT[p]
                kt_ = qkT[4 + p]
                nc0 = ng * 1024 + half * 512
                es = []

                def gen():
                    e2 = None
                    for mt in range(NT):
                        sp = spsum.tile([128, 1024], f32, tag="sps", name="sp")
                        for hx in range(1 if probe == "s_half" else 2):
                            po = hx * 64
                            nc.tensor.matmul(
                                sp[:, hx * 512:(hx + 1) * 512],
                                lhsT=kt_[po:po + 64, mt * 128:(mt + 1) * 128],
                                rhs=qt[po:po + 64, nc0:nc0 + 512],
                                start=True,
                                stop=True,
                            )
                        if pv_mdr:
                            if mt % 2 == 0:
                                e2 = workp.tile([128, 2048], e_dt, tag="e",
                                                name="e")
                            e = e2[:, (mt % 2) * 1024:(mt % 2) * 1024 + 1024]
                        else:
                            e = workp.tile([128, 1024], e_dt, tag="e", name="e")
                        if mt in dve_set:
                            c0, c1, c2, c3 = PCOEF
                            add, mul = (mybir.AluOpType.add,
                                        mybir.AluOpType.mult)
                            y = polyp.tile([128, 1024], bf16, tag="py")
                            t = polyp.tile([128, 1024], bf16, tag="pt")
                            u = polyp.tile([128, 1024], bf16, tag="pu")
                            nc.vector.tensor_scalar_mul(y, sp, PALPHA * 2.0)
                            nc.vector.scalar_tensor_tensor(
                                out=t, in0=y, scalar=c0, in1=y, op0=add, op1=mul)
                            nc.vector.scalar_tensor_tensor(
                                out=u, in0=t, scalar=c1, in1=y, op0=add, op1=mul)
                            nc.vector.scalar_tensor_tensor(
                                out=t, in0=u, scalar=c2, in1=y, op0=add, op1=mul)
                            nc.vector.tensor_scalar_add(u, t, c3)
                            nc.vector.tensor_mul(out=e, in0=u, in1=u)
                        else:
                            if probe == "exp_half":
                                nc.scalar.activation(
                                    out=e[:, 0:512], in_=sp[:, 0:512],
                                    func=mybir.ActivationFunctionType.Exp,
                                    scale=0.125,
                                )
                            else:
                                nc.scalar.activation(
                                    out=e, in_=sp,
                                    func=mybir.ActivationFunctionType.Exp,
                                    scale=0.125,
                                )
                        es.append(e)
                        if fillers and mt in slots:
                            fillers.pop(0)()
                        yield
                    while fillers:
                        fillers.pop(0)()

                return es, gen()

            def pv_units_pair(blk, es, fillers=(), pe_norm=True):
                """Pair mode PV: accumulators pv[0]/pv[1] for heads 2p/2p+1;
                e columns 0:512 belong to head 2p, 512:1024 to 2p+1."""
                p, ng, half = blk
                fillers = list(fillers)
                at = attnT[p]
                nc0 = ng * 1024 + half * 512
                pvs = [pvpsum.tile([65, 512], f32, tag="pv", name="pv")
                       for _ in range(2)]

                def norm(hx):
                    pv = pvs[hx]
                    r = smallp.tile([1, 512], bf16, tag="r", name="r")
                    with nc.allow_low_precision(reason="softmax recip bcast"):
                        nc.vector.reciprocal(out=r, in_=pv[64:65, :])
                    pvsb = smallp.tile([64, 512], f32, tag="pvsb", name="pvsb")
                    nc.vector.tensor_copy(out=pvsb, in_=pv[0:64, :])
                    if pe_norm:
                        rbs = smallp.tile([64, 512], f32, tag="rbs", name="rbs")
                        rb = mmp.tile([64, 512], f32, tag="mm", name="rb")
                        nc.tensor.matmul(rb, lhsT=ones, rhs=r,
                                         start=True, stop=True)
                        nc.vector.tensor_copy(out=rbs, in_=rb)
                    else:
                        rbs = smallp.tile([64, 512], bf16, tag="rbs", name="rbs")
                        idx = rb_idx[0] % 32
                        rb_idx[0] += 1
                        nc.sync.dma_start(out=rdram[idx], in_=r[0, :])
                        rsl = rdram[idx]
                        bcast = bass.AP(tensor=rsl.tensor, offset=rsl.offset,
                                        ap=[[0, 64]] + [list(d) for d in rsl.ap])
                        nc.sync.dma_start(out=rbs, in_=bcast)
                    nc.vector.tensor_mul(
                        out=at[hx * 64:hx * 64 + 64, nc0:nc0 + 512],
                        in0=pvsb,
                        in1=rbs,
                    )

                def gen_mdr():
                    h0 = 2 * p
                    last_t = NT // 2 - (2 if probe == "pv_half" else 1)
                    for t in range(NT // 2):
                        if fillers:
                            fillers.pop(0)()
                        if probe == "pv_half" and t % 2 == 1:
                            continue
                        for hx in range(2):
                            h = h0 + hx
                            nc.tensor.matmul(
                                pvs[hx],
                                lhsT=v1[t][:, :, h * 68:h * 68 + 65],
                                rhs=plane_ap(
                                    es[2 * t][:, hx * 512:hx * 512 + 512],
                                    1024),
                                start=(t == 0),
                                stop=(t == last_t),
                                perf_mode=DR,
                            )
                            yield
                    norm(0)
                    norm(1)

                def gen():
                    h0 = 2 * p
                    last_mt = NT - 2 if probe == "pv_half" else NT - 1
                    for mt in range(NT):
                        if fillers:
                            fillers.pop(0)()
                        if probe == "pv_half" and mt % 2 == 1:
                            continue
                        for hx in range(2):
                            h = h0 + hx
                            ecol = 0 if probe == "exp_half" else hx * 512
                            if pv_dr:
                                nc.tensor.matmul(
                                    pvs[hx],
                                    lhsT=v1[mt][:, :, h * 68:h * 68 + 65],
                                    rhs=dup_plane(
                                        es[mt][:, ecol:ecol + 512]),
                                    start=(mt == 0),
                                    stop=(mt == last_mt),
                                    perf_mode=DR,
                                )
                            else:
                                nc.tensor.matmul(
                                    pvs[hx],
                                    lhsT=v1[mt][:, h * 65:(h + 1) * 65],
                                    rhs=es[mt][:, ecol:ecol + 512],
                                    start=(mt == 0),
                                    stop=(mt == last_mt),
                                )
                            yield
                    norm(0)
                    norm(1)

                return gen_mdr() if pv_mdr else gen()

            def s_units(h, ng, fillers, dense=False):
                """Generator of 16 S-phase units for block (h, ng): each emits
                the two S matmuls + the exp, plus an optional filler group.
                Appends e tiles to the returned list as units run.  With
                `dense`, one filler is popped at every mt (slot i == mt i, for
                fillers that must land before a specific S/PV consumer)."""
                fillers = list(fillers)
                if dense:
                    slots = set(range(NT))
                elif fillers:
                    stride = max(1, NT // len(fillers))
                    slots = set(range(stride - 1, NT, stride))
                else:
                    slots = set()
                qt = qkT[h // 2]
                kt_ = qkT[4 + h // 2]
                po = (h % 2) * 64
                es = []

                def gen():
                    for mt in range(NT):
                        sp = spsum.tile([128, 1024], f32, tag="sps", name="sp")
                        for half in range(2):
                            lhsT = kt_[po:po + 64, mt * 128:(mt + 1) * 128]
                            rhs = qt[po:po + 64,
                                     ng * 1024 + half * 512:
                                     ng * 1024 + (half + 1) * 512]
                            if s_dr:
                                lhsT, rhs = dup_plane(lhsT), dup_plane(rhs)
                            nc.tensor.matmul(
                                sp[:, half * 512:(half + 1) * 512],
                                lhsT=lhsT,
                                rhs=rhs,
                                start=True,
                                stop=True,
                                perf_mode=DR if s_dr else None,
                            )
                        e = workp.tile([128, 1024], e_dt, tag="e", name="e")
                        if mt in dve_set:
                            # DVE/GpSimd polynomial exp: frees ScalarE cycles
                            c0, c1, c2, c3 = PCOEF
                            add, mul = (mybir.AluOpType.add,
                                        mybir.AluOpType.mult)
                            y = polyp.tile([128, 1024], bf16, tag="py")
                            t = polyp.tile([128, 1024], bf16, tag="pt")
                            u = polyp.tile([128, 1024], bf16, tag="pu")
                            nc.vector.tensor_scalar_mul(
                                y, sp, PALPHA * (1.0 if s_dr else 2.0))
                            nc.vector.scalar_tensor_tensor(
                                out=t, in0=y, scalar=c0, in1=y, op0=add, op1=mul)
                            nc.vector.scalar_tensor_tensor(
                                out=u, in0=t, scalar=c1, in1=y, op0=add, op1=mul)
                            nc.vector.scalar_tensor_tensor(
                                out=t, in0=u, scalar=c2, in1=y, op0=add, op1=mul)
                            nc.vector.tensor_scalar_add(u, t, c3)
                            nc.vector.tensor_mul(out=e, in0=u, in1=u)
                        else:
                            nc.scalar.activation(
                                out=e, in_=sp,
                                func=mybir.ActivationFunctionType.Exp,
                                scale=0.0625 if s_dr else 0.125,
                            )
                        es.append(e)
                        if fillers and mt in slots:
                            fillers.pop(0)()
                        yield
                    while fillers:  # leftovers
                        fillers.pop(0)()

                return es, gen()

            def pv_units(h, ng, es, fillers=(), pe_norm=False):
                """Generator of 32 PV matmul units for block (h, ng); after
                exhaustion emits the two normalization chains.  `fillers` are
                popped one per mt (used to finish V tiles ahead of their PV
                use).  `pe_norm` broadcasts 1/denom with a PE matmul instead
                of the DRAM bounce (shorter latency; used for late blocks on
                the critical path to proj)."""
                fillers = list(fillers)
                po = (h % 2) * 64
                at = attnT[h // 2]
                pvs = [pvpsum.tile([65, 512], f32, tag="pv", name="pv")
                       for _ in range(2)]

                def norm(half):
                    pv = pvs[half]
                    r = smallp.tile([1, 512], bf16, tag="r", name="r")
                    with nc.allow_low_precision(reason="softmax recip bcast"):
                        nc.vector.reciprocal(out=r, in_=pv[64:65, :])
                    # copy the numerator out of PSUM immediately so the pv
                    # slot frees for the next block's PV without waiting for
                    # the broadcast round-trip
                    pvsb = smallp.tile([64, 512], f32, tag="pvsb", name="pvsb")
                    nc.vector.tensor_copy(out=pvsb, in_=pv[0:64, :])
                    rbs = smallp.tile([64, 512], f32, tag="rbs", name="rbs")
                    if pe_norm:
                        rb = mmp.tile([64, 512], f32, tag="mm", name="rb")
                        nc.tensor.matmul(rb, lhsT=ones, rhs=r,
                                         start=True, stop=True)
                        nc.vector.tensor_copy(out=rbs, in_=rb)
                    else:
                        idx = rb_idx[0] % 32
                        rb_idx[0] += 1
                        nc.sync.dma_start(out=rdram[idx], in_=r[0, :])
                        rsl = rdram[idx]
                        bcast = bass.AP(tensor=rsl.tensor, offset=rsl.offset,
                                        ap=[[0, 64]] + [list(p) for p in rsl.ap])
                        nc.sync.dma_start(out=rbs, in_=bcast)
                    nc.vector.tensor_mul(
                        out=at[po:po + 64,
                               ng * 1024 + half * 512:
                               ng * 1024 + (half + 1) * 512],
                        in0=pvsb,
                        in1=rbs,
                    )

                def gen():
                    for mt in range(NT):
                        if fillers:
                            fillers.pop(0)()
                        for half in range(2):
                            if pv_dr:
                                nc.tensor.matmul(
                                    pvs[half],
                                    lhsT=v1[mt][:, :, h * 68:h * 68 + 65],
                                    rhs=dup_plane(
                                        es[mt][:, half * 512:(half + 1) * 512]),
                                    start=(mt == 0),
                                    stop=(mt == NT - 1),
                                    perf_mode=DR,
                                )
                            else:
                                nc.tensor.matmul(
                                    pvs[half],
                                    lhsT=v1[mt][:, h * 65:(h + 1) * 65],
                                    rhs=es[mt][:, half * 512:(half + 1) * 512],
                                    start=(mt == 0),
                                    stop=(mt == NT - 1),
                                )
                            yield
                    norm(0)
                    norm(1)

                return gen()

            def run_all(g):
                for _ in g:
                    pass

            def interleave(sgen, pvgen, npv=2):
                """npv PV units per S unit (32 or 16 PV vs 16 S per block)."""
                while True:
                    done = 0
                    for _ in range(npv):
                        if next(pvgen, StopIteration) is StopIteration:
                            done += 1
                            break
                    if next(sgen, StopIteration) is StopIteration:
                        done += 1
                    if done:
                        for _ in pvgen:
                            pass
                        for _ in sgen:
                            pass
                        return

            def proj(nt):
                # the second half of proj runs after the last attention block:
                # rotate over the then-idle pv/sps PSUM slots too, so groups
                # aren't serialized on the two "mm" slots
                if nt < 8:
                    pool_tag = (mmp, "mm")
                elif early_proj and nt < 12:
                    # interleaved with the final PV drain: pvpsum slots are
                    # still held by live accumulators (a proj tile request
                    # there would deadlock the PE queue)
                    pool_tag = [(mmp, "mm"), (spsum, "sps")][nt % 2]
                else:
                    pool_tag = [(mmp, "mm"), (pvpsum, "pv"), (spsum, "sps")][nt % 3]
                ot = outp.tile([128, C], f32, tag="ot")
                for ob in range(2):
                    ps = pool_tag[0].tile([128, 512], f32, tag=pool_tag[1])
                    for ct in range(QKC // 128):
                        nc.tensor.matmul(
                            ps,
                            lhsT=attnT[ct][:, nt * 128:(nt + 1) * 128],
                            rhs=wp[:, ct, ob * 512:(ob + 1) * 512],
                            start=(ct == 0),
                            stop=(ct == QKC // 128 - 1),
                        )
                    # ScalarE is idle during the projection tail; DVE is not.
                    # Under early_proj the first 8 projs run while ScalarE is
                    # still the softmax bottleneck -> their copies go to DVE.
                    if early_proj and nt < 8:
                        nc.vector.tensor_copy(
                            out=ot[:, ob * 512:(ob + 1) * 512], in_=ps
                        )
                    else:
                        nc.scalar.copy(
                            out=ot[:, ob * 512:(ob + 1) * 512], in_=ps
                        )
                nc.sync.dma_start(
                    out=out_d[nt * 128:(nt + 1) * 128, :], in_=ot
                )

            # Software pipeline over 16 (h, ng) blocks: block i's S-phase (the
            # exp feed) interleaves with block i-1's PV matmuls so ScalarE
            # never starves at head boundaries.  Only qk tiles 0 and 4 precede
            # attention; V tiles are built as fillers inside blocks 0/1, later
            # qk tile-groups inside earlier pairs' blocks (always complete
            # before first use).  The last block's PV overlaps the output
            # projection, and the last two blocks normalize via PE broadcast
            # (short latency) instead of the DRAM bounce.
            import functools
            for _rep in range(repeat):
              # Minimal prelude: S(b0=(h0,ng0), mt) needs q cols 0:1024
              # (qk groups (0,0),(0,1)) and k block nb0 (group (4,0)); the
              # other qk(0)/qk(4) groups and V tiles ride as dense fillers
              # inside block 0, ordered so each lands before its first
              # consumer (group (4,j) before S mt=4j; v1[i] before PV mt i).
              qk_group(0, 0)
              qk_group(0, 1)
              qk_group(4, 0)

              qkg = [[functools.partial(qk_group, m, nb) for nb in range(NB)]
                   for m in range(QKT)]
              vg = [functools.partial(v_tile, nt) for nt in range(NT)]
              block_fill = [[] for _ in range(16)]
              block_fill[0] = [
                  qkg[4][1], qkg[0][2], qkg[0][3], vg[0],
                  qkg[4][2], vg[1], vg[2], vg[3],
                  qkg[4][3], vg[4], vg[5], vg[6],
                  vg[7], vg[8], vg[9], vg[10],
              ]
              pv0_fill = vg[11:16]       # v1[11..15] paced inside PV(b0)
              f15 = qkg[1] + qkg[5]      # tiles 1,5 for head pair 1 (blocks 4-7)
              block_fill[2] = f15[0:4]
              block_fill[3] = f15[4:8]
              f26 = qkg[2] + qkg[6]      # tiles 2,6 for pair 2 (blocks 8-11)
              for i in range(4):
                  block_fill[4 + i] = f26[2 * i:2 * i + 2]
              f37 = qkg[3] + qkg[7]      # tiles 3,7 for pair 3 (blocks 12-15)
              for i in range(4):
                  block_fill[8 + i] = f37[2 * i:2 * i + 2]

              if pair_s:
                  if early_proj:
                      # blocks 14/15 have no qk fillers; proj rows for n<1024
                      # only need norms from blocks <= 13 (done by then), so
                      # they ride the late S-phases' filler slots
                      pf = [functools.partial(proj, nt) for nt in range(8)]
                      block_fill[14] = pf[0:4]
                      block_fill[15] = pf[4:8]
                  blocks = [(p, ng, hf) for p in range(4) for ng in range(2)
                            for hf in range(2)]
                  prev_pv = None
                  for bi, blk in enumerate(blocks):
                      es, sgen = s_units_pair(blk, block_fill[bi],
                                              dense=(bi == 0))
                      if prev_pv is None:
                          run_all(sgen)
                      else:
                          # Drain the previous block's PV (16 units in mdr)
                          # entirely at the START of the S phase: frees the
                          # pvpsum slots + norm chain earliest and leaves the
                          # S pairs to run back-to-back (uninterrupted
                          # row-group overlap).  npv 16 >= 2 > 1, measured.
                          interleave(sgen, prev_pv,
                                     npv=cfg.get("npv_override", 16))
                      prev_pv = pv_units_pair(
                          blk, es,
                          fillers=pv0_fill if bi == 0 else (),
                          pe_norm=(not dma_norm) or bi >= 14,
                      )
              else:
                blocks = [(h, ng) for h in range(HPC) for ng in range(2)]
                prev_pv = None
                for bi, (h, ng) in enumerate(blocks):
                  es, sgen = s_units(h, ng, block_fill[bi], dense=(bi == 0))
                  if prev_pv is None:
                      run_all(sgen)
                  else:
                      interleave(sgen, prev_pv)
                  prev_pv = pv_units(
                      h, ng, es,
                      fillers=pv0_fill if bi == 0 else (),
                      pe_norm=True,
                  )

              # tail: last block's PV interleaved with the next projection
              # slice (whose norms are already done); then its norm (PE
              # broadcast, short), then the remaining projection.
              tail0 = 8 if early_proj else 0
              for nt in range(tail0, tail0 + 4 if early_proj else 8):
                  for _ in range(4):
                      next(prev_pv, None)
                  proj(nt)
              run_all(prev_pv)
              for nt in range(tail0 + 4 if early_proj else 8, NT):
                  proj(nt)

    if split_waits:
        _split_multi_waits(nc, mybir)
    return nc


def _split_multi_waits(nc, mybir):
    """TPB instructions carry exactly one sync-wait slot; walrus codegen
    rejects instructions Tile scheduled with >1 waits ("Too many sync wait
    commands").  Hoist all but the last wait onto NoOps inserted just before
    the instruction on the same engine queue (queues execute in order, so
    semantics are identical)."""
    eng_ok = {
        mybir.EngineType.PE,
        mybir.EngineType.Activation,
        mybir.EngineType.DVE,
        mybir.EngineType.Pool,
        mybir.EngineType.SP,
    }
    k = 0
    for f in nc.m.functions:
        for blk in f.blocks:
            out = []
            changed = False
            for inst in blk.instructions:
                si = inst.sync_info
                if (
                    si is not None
                    and len(si.on_wait) > 1
                    and inst.engine in eng_ok
                ):
                    waits = list(si.on_wait)
                    for w in waits[:-1]:
                        nop = mybir.InstNoOp(name=f"I-splitw-{k}", ins=[], outs=[])
                        k += 1
                        nop.engine = inst.engine
                        nop.sync_info = mybir.SyncInfo(on_wait=[w], on_update=[])
                        out.append(nop)
                    inst.sync_info = mybir.SyncInfo(
                        on_wait=[waits[-1]], on_update=list(si.on_update)
                    )
                    changed = True
                out.append(inst)
            if changed:
                blk.instructions = out


def _get_nc():
    global _nc_cache
    if _nc_cache is None:
        _nc_cache = build_nc()
    return _nc_cache


def make_in_maps(x, W_qkv, W_proj):
    bf16 = ml_dtypes.bfloat16
    in_maps = []
    for c in range(NCORES):
        b, g = divmod(c, 2)
        xT = np.ascontiguousarray(np.asarray(x[b]).T).astype(bf16)
        wq = W_qkv[:, g * QKC:(g + 1) * QKC]
        wk = W_qkv[:, C + g * QKC:C + (g + 1) * QKC]
        # pair-major: [q128 | k128] per head pair, matching qk_group's co map
        wqk = np.concatenate(
            [blk for p in range(4)
             for blk in (wq[:, p * 128:(p + 1) * 128],
                         wk[:, p * 128:(p + 1) * 128])],
            axis=1,
        ).astype(bf16)
        wv = np.ascontiguousarray(W_qkv[:, 2 * C + g * QKC:2 * C + (g + 1) * QKC]).astype(bf16)
        wp = np.ascontiguousarray(W_proj[g * QKC:(g + 1) * QKC, :]).astype(bf16)
        in_maps.append({"xT": xT, "wqk": wqk, "wv": wv, "wp": wp})
    return in_maps


last_exec_time_ns = None


def kernel(x, W_qkv, W_proj, b_proj):
    global last_exec_time_ns
    import os
    # the NTFF trace path needs antenv.axon_hooks, absent in this container
    os.environ["BASS_NEVER_TRACE"] = "1"
    from concourse import bass_utils

    x = np.asarray(x)
    W_qkv = np.asarray(W_qkv)
    W_proj = np.asarray(W_proj)
    b_proj = np.asarray(b_proj)

    nc = _get_nc()
    in_maps = make_in_maps(x, W_qkv, W_proj)
    res = bass_utils.run_bass_kernel_spmd(nc, in_maps, list(range(NCORES)))
    last_exec_time_ns = res.exec_time_ns

    out = np.empty((B, N, C), np.float32)
    bias = b_proj.astype(np.float32)
    for b in range(B):
        out[b] = res.results[2 * b]["out"] + res.results[2 * b + 1]["out"] + bias
    return out

